# revision 60
# baseline (speedup 1.0000x reference)
"""Expert-parallel MoE kernel for Trainium2 (8 NeuronCores), host-dispatched.

Problem: top-2-of-8 MoE layer, H=768, F=3072, T=2048 tokens, fp32.

Sharding strategy (per the expert-parallel hint): the router is tiny
(T x H x E = 12.6 MFLOP), so routing runs on host as part of input
sharding. Each core e receives ONLY the tokens routed to expert e
(gathered, zero-padded to a common capacity C), plus expert e's FFN
weights in bf16. The device kernel computes the dense FFN
  y = gelu(x @ w1.T + b1) @ w2.T
over its C tokens. The host then scatter-adds g * (y + b2) into the
full [T, H] output (the "combine" step of the sharding). This removes
the 4x excess compute of evaluating every expert on every token.

Load balancing: experts are paired heavy-with-light; each pair gets two
cores, each computing HALF the FFN (a split along F) for BOTH experts
of its pair. GELU is elementwise per F column, so the split is exact;
the host sums the two partial GEMM2 outputs. Per-core work drops from
max_e(n_e) to (max lefts + max rights)/2 token-equivalents with the
same weight-DMA bytes (half-F x two experts).

Device layout: activations travel with tokens on the free axis and
hidden/ffn dims on partitions, so both GEMMs run at 1 cycle/row (bf16)
with no ceil(C/128) padding waste in GEMM2.
"""

import numpy as np

import concourse.bass as bass
import concourse.mybir as mybir
import concourse.tile as tile
from concourse import bacc
from concourse.bass_utils import run_bass_kernel_spmd

E = 8
H = 768
F = 3072
B, S = 2, 1024
T = B * S
HC = H // 128         # 6 H chunks (GEMM1 contraction / GEMM2 output rows)
FC = F // 128          # 24 F chunks (GEMM1 output rows / GEMM2 contraction)
FH = F // 2            # F half per core (pair-wise F-split)
FCH = FC // 2          # 12 F chunks per expert segment

f32 = mybir.dt.float32
bf16 = mybir.dt.bfloat16
AF = mybir.ActivationFunctionType
OP = mybir.AluOpType


def _token_chunks(C):
    """Split C tokens into equal-ish chunks of <=512 (PSUM bank limit)."""
    n = -(-C // 512)
    base = C // n
    rem = C - base * n
    return [base + (1 if i < rem else 0) for i in range(n)]


def build_nc(CL, CR):
    C = CL + CR
    # segment s covers tokens [soff, soff+slen) using fc chunks
    # [s*FCH, (s+1)*FCH) of the packed half-F weights
    segs = []
    off = 0
    for s, slen in enumerate((CL, CR)):
        segs.append((s, off, _token_chunks(slen)))
        off += slen
    nc = bacc.Bacc("TRN2", target_bir_lowering=False, debug=False)

    xT = nc.dram_tensor("xT", [128, HC * C], bf16, kind="ExternalInput")
    w1p = nc.dram_tensor("w1p", [128, FC * HC * 128], bf16, kind="ExternalInput")
    w2p = nc.dram_tensor("w2p", [128, FC * H], bf16, kind="ExternalInput")
    b1c = nc.dram_tensor("b1c", [128, FC], f32, kind="ExternalInput")
    yout = nc.dram_tensor("yout", [128, HC * C], f32, kind="ExternalOutput")

    with tile.TileContext(nc) as tc:
        with (
            tc.tile_pool(name="wpool", bufs=1) as wpool,
            tc.tile_pool(name="hpool", bufs=FC + 2) as hpool,
            tc.tile_pool(name="ypool", bufs=8) as ypool,
            tc.tile_pool(name="ps1", bufs=3, space="PSUM") as ps1,
            tc.tile_pool(name="psy", bufs=4, space="PSUM") as psy,
        ):
            b1sb = wpool.tile([128, FC], f32, tag="b1sb")
            w1sb = wpool.tile([128, FC, HC * 128], bf16, tag="w1sb")
            w2sb = wpool.tile([128, HC, FC * 128], bf16, tag="w2sb")
            flat_chunks = [tch for _, _, chs in segs for tch in chs]
            xcs = [
                wpool.tile([128, HC, tch], bf16, tag="xc", name=f"xc_{ci}")
                for ci, tch in enumerate(flat_chunks)
            ]

            # xT is packed per-chunk on host: chunk ci occupies columns
            # [HC*t0, HC*(t0+tch)) laid out [HC, tch], so each chunk (and
            # each k-slice of chunk 0) is one contiguous DMA.
            xflat = xT.ap()
            w1v = w1p.ap().rearrange("p (c x) -> p c x", c=FC)
            w2v = w2p.ap().rearrange("p (c x) -> p c x", c=HC)
            tch0 = flat_chunks[0]

            # --- head ------------------------------------------------------
            # First-compute critical path: every input DMA pays ~2.2us of
            # DGE + completion-semaphore latency, so the two operands of
            # the first matmul go first on separate queues. A tiny warm
            # gelu (on a memset scratch, no DMA dependency) preloads the
            # Act Gelu table before the first real gelu.
            scratch = wpool.tile([128, 1], f32, tag="scratch")
            warm = wpool.tile([128, 1], f32, tag="warm")
            nc.vector.memset(scratch[:], 0.0)
            # SP: w1[0] k=0 slice and x0[k=0] — the two operands of the
            # very first matmul — then the rest of w1[0]. (The Act queue is
            # blocked ~1.3us at start by the preamble act-table load, so
            # head-critical DMAs must avoid it.)
            nc.sync.dma_start(w1sb[:, 0, 0:128], w1v[:, 0, 0:128])
            nc.sync.dma_start(xcs[0][:, 0, :], xflat[:, 0:tch0])
            nc.sync.dma_start(w1sb[:, 0, 128:], w1v[:, 0, 128:])
            # Act: x0[k=1], b1, then the Gelu-table warm load.
            nc.scalar.dma_start(xcs[0][:, 1, :], xflat[:, tch0 : 2 * tch0])
            nc.scalar.dma_start(b1sb[:], b1c.ap())
            nc.scalar.activation(warm[:], scratch[:], AF.Gelu)
            # Pool: remaining chunk-0 x slices, concurrent with SP's
            # weight stream.
            for k in range(2, HC):
                nc.gpsimd.dma_start(
                    xcs[0][:, k, :], xflat[:, k * tch0 : (k + 1) * tch0]
                )
            # Weight stream on SP: the rest of w1 (GEMM1 phase order), then
            # w2 as hc-major blocks (GEMM2 group order). w1 goes as two
            # singletons then pairs: SP's DGE issue overhead (~625ns) is as
            # long as a single fc transfer, so per-fc DMAs would be
            # issue-bound and fall behind the PE.
            w1flat = w1p.ap()
            fcw = HC * 128
            fhw = FCH * 128

            def w1_stream(fc_from, fc_to, first_singles):
                fc = fc_from
                while fc < fc_to:
                    n = 1 if (first_singles and fc <= fc_from + 1) else min(
                        2, fc_to - fc
                    )
                    nc.sync.dma_start(
                        w1sb[:, fc : fc + n, :],
                        w1flat[:, fc * fcw : (fc + n) * fcw].rearrange(
                            "p (c x) -> p c x", c=n
                        ),
                    )
                    fc += n

            # Stream follows consumption order: seg-0 w1, seg-0 halves of
            # the w2 hc-blocks, seg-1 w1, seg-1 w2 halves.
            w1_stream(1, FCH, True)
            for hc in range(HC):
                nc.sync.dma_start(
                    w2sb[:, hc, 0:fhw], w2v[:, hc, 0:fhw]
                )
            w1_stream(FCH, FC, False)
            for hc in range(HC):
                nc.sync.dma_start(
                    w2sb[:, hc, fhw:], w2v[:, hc, fhw:]
                )
            off = HC * tch0
            for ci in range(1, len(flat_chunks)):
                tch = flat_chunks[ci]
                nc.gpsimd.dma_start(
                    xcs[ci][:],
                    xflat[:, off : off + HC * tch].rearrange(
                        "p (c t) -> p c t", c=HC
                    ),
                )
                off += HC * tch

            youtv = yout.ap().rearrange("p (c t) -> p c t", c=HC)

            ci = -1
            for seg, soff, schunks in segs:
                coff = 0
                for sci, tch in enumerate(schunks):
                    ci += 1
                    xsb = xcs[ci]
                    t0 = soff + coff
                    coff += tch
                    # --- GEMM1 phase: this segment's FCH chunks, gelu
                    # trails on Act ---------------------------------------
                    hms = []
                    for fcl in range(FCH):
                        fc = seg * FCH + fcl
                        hps = ps1.tile([128, tch], f32, tag="hps")
                        for k in range(HC):
                            nc.tensor.matmul(
                                hps[:],
                                w1sb[:, fc, bass.ts(k, 128)],
                                xsb[:, k, :],
                                start=(k == 0),
                                stop=(k == HC - 1),
                            )
                        hm = hpool.tile([128, tch], bf16, tag="hm")
                        nc.scalar.activation(
                            hm[:], hps[:], AF.Gelu, bias=b1sb[:, fc : fc + 1]
                        )
                        hms.append(hm)
                    # --- GEMM2 phase, grouped per hc so each yps closes
                    # early and the PSUM drain overlaps the rest ----------
                    last = seg == len(segs) - 1 and sci == len(schunks) - 1
                    for hc in range(HC):
                    # The very last group closes in two half-token PSUM
                    # tiles so most of its drain overlaps the final matmuls.
                        halves = (
                            [(0, tch - 64), (tch - 64, tch)]
                            if (last and hc == HC - 1 and tch > 128)
                            else [(0, tch)]
                        )
                        for h0, h1 in halves:
                            yps = psy.tile([128, h1 - h0], f32, tag="yps")
                            for fcl in range(FCH):
                                fc = seg * FCH + fcl
                                nc.tensor.matmul(
                                    yps[:],
                                    w2sb[:, hc, bass.ts(fc, 128)],
                                    hms[fcl][:, h0:h1],
                                    start=(fcl == 0),
                                    stop=(fcl == FCH - 1),
                                )
                            ysb = ypool.tile([128, h1 - h0], f32, tag="ysb")
                            # Two-wide drain: evicts alternate DVE / Act
                            # (Copy is in the gelu table set, so no reload;
                            # GPSIMD cannot read PSUM), output DMAs
                            # alternate SP/Pool.
                            if hc % 2 == 0:
                                nc.vector.tensor_scalar(
                                    ysb[:], yps[:], 1.0, None, op0=OP.mult
                                )
                            else:
                                nc.scalar.activation(ysb[:], yps[:], AF.Copy)
                            # Final piece drains via SP: its HWDGE fixed
                            # cost (~625ns) beats Pool's SWDGE (~994ns) on
                            # the kernel-exit critical path.
                            if last and hc == HC - 1 and h1 == tch:
                                deng = nc.sync
                            else:
                                deng = nc.sync if hc % 2 == 0 else nc.gpsimd
                            deng.dma_start(
                                youtv[:, hc, t0 + h0 : t0 + h1], ysb[:]
                            )
    nc.compile()
    return nc


_NC_CACHE = {}


def _get_nc(CL, CR):
    if (CL, CR) not in _NC_CACHE:
        _NC_CACHE[(CL, CR)] = build_nc(CL, CR)
    return _NC_CACHE[(CL, CR)]


def _chunk_partition(a, nchunks):
    """[nchunks*128, X] -> [128, nchunks*X] (chunk-of-rows onto partitions)."""
    n, x = a.shape
    return np.ascontiguousarray(
        a.reshape(nchunks, 128, x).transpose(1, 0, 2).reshape(128, nchunks * x)
    )


def _prepare(hidden_states, router_w, w1, b1, w2, b2):
    x = np.asarray(hidden_states, dtype=np.float32).reshape(T, H)
    router_w = np.asarray(router_w, dtype=np.float32)
    w1 = np.asarray(w1, dtype=np.float32)
    b1 = np.asarray(b1, dtype=np.float32)
    w2 = np.asarray(w2, dtype=np.float32)
    b2 = np.asarray(b2, dtype=np.float32)

    # Router on host (part of dispatch): exact top-2 + softmax, matching
    # jax.lax.top_k tie-breaking (stable order on descending sort).
    logits = x @ router_w.T                                   # [T, E]
    top_idx = np.argsort(-logits, axis=1, kind="stable")[:, :2]
    tv = np.take_along_axis(logits, top_idx, axis=1).astype(np.float64)
    ex = np.exp(tv - tv[:, :1])
    probs = ex / ex.sum(axis=1, keepdims=True)                # [T, 2]

    toks, gates = [], []
    for e in range(E):
        m = top_idx == e
        tok = np.nonzero(m.any(axis=1))[0]
        toks.append(tok)
        gates.append((probs * m)[tok].sum(axis=1))

    # Pair heavy-with-light: the 4 largest experts are "left" segments,
    # the 4 smallest "right". Capacity = max(lefts) + max(rights).
    order = sorted(range(E), key=lambda e: -len(toks[e]))
    lefts, rights = order[:4], order[4:]
    pairs = list(zip(lefts, rights))
    CL = max(128, max(len(toks[e]) for e in lefts))
    CR = max(128, max(len(toks[e]) for e in rights))

    nc = _get_nc(CL, CR)

    def pack_x(e, cap):
        tok = toks[e]
        xe = np.zeros((cap, H), dtype=np.float32)
        xe[: len(tok)] = x[tok]
        blocks, t0 = [], 0
        for tch in _token_chunks(cap):
            blocks.append(
                _chunk_partition(np.ascontiguousarray(xe[t0 : t0 + tch].T), HC)
            )
            t0 += tch
        return blocks

    def w1_half(e, h):
        w1h = w1[e][h * FH : (h + 1) * FH, :]                  # [FH, H]
        w1t = _chunk_partition(np.ascontiguousarray(w1h.T), HC)  # [128,HC*FH]
        return (
            w1t.reshape(128, HC, FCH, 128)
            .transpose(0, 2, 1, 3)
            .reshape(128, FCH * HC * 128)
        )

    def w2_half(e, h):
        w2h = w2[e][:, h * FH : (h + 1) * FH]                  # [H, FH]
        w2t = _chunk_partition(np.ascontiguousarray(w2h.T), FCH)  # [128,FCH*H]
        return (
            w2t.reshape(128, FCH, HC, 128)
            .transpose(0, 2, 1, 3)
            .reshape(128, HC, FCH * 128)
        )

    bf = mybir.dt.np(bf16)
    in_maps = [None] * E
    for p, (eL, eR) in enumerate(pairs):
        xTe = np.concatenate(pack_x(eL, CL) + pack_x(eR, CR), axis=1).astype(bf)
        for h in range(2):
            w1pe = np.concatenate(
                [w1_half(eL, h), w1_half(eR, h)], axis=1
            ).astype(bf)
            w2pe = np.ascontiguousarray(
                np.concatenate([w2_half(eL, h), w2_half(eR, h)], axis=2)
            ).reshape(128, HC * FC * 128).astype(bf)
            b1ce = np.concatenate(
                [
                    b1[eL][h * FH : (h + 1) * FH].reshape(FCH, 128).T,
                    b1[eR][h * FH : (h + 1) * FH].reshape(FCH, 128).T,
                ],
                axis=1,
            )
            b1ce = np.ascontiguousarray(b1ce)
            in_maps[2 * p + h] = {
                "xT": xTe, "w1p": w1pe, "w2p": w2pe, "b1c": b1ce,
            }

    return nc, in_maps, (toks, gates, b2, pairs, CL, CR)


def kernel(hidden_states, router_w, w1, b1, w2, b2):
    nc, in_maps, (toks, gates, b2, pairs, CL, CR) = _prepare(
        hidden_states, router_w, w1, b1, w2, b2
    )

    global _last_nc, _last_in_maps
    _last_nc, _last_in_maps = nc, in_maps
    res = run_bass_kernel_spmd(nc, in_maps, core_ids=list(range(E)))

    C = CL + CR
    out = np.zeros((T, H), dtype=np.float64)
    for p, (eL, eR) in enumerate(pairs):
        y = np.zeros((C, H), dtype=np.float64)
        for h in range(2):
            y += (
                res.results[2 * p + h]["yout"]
                .reshape(128, HC, C)
                .transpose(2, 1, 0)
                .reshape(C, H)
                .astype(np.float64)
            )
        for e, y0 in ((eL, 0), (eR, CL)):
            tok = toks[e]
            if len(tok) == 0:
                continue
            out[tok] += gates[e][:, None] * (
                y[y0 : y0 + len(tok)] + b2[e][None, :]
            )
    return out.reshape(B, S, H).astype(np.float32)
